# revision 1
# baseline (speedup 1.0000x reference)
"""KAN layer on 8 Trainium2 NeuronCores (Bass/Tile).

Computes out = x @ base_weight.T + silu(x) @ spline_weight.sum(-1).T
for x:[8192,1024] f32, base_weight:[1024,1024] f32,
spline_weight:[1024,1024,8] f32 -> out:[8192,1024] f32.

Strategy (self-contained, hardcoded for these shapes):
  * 2D shard over the 8 cores: batch split R=2, out-features split C=4.
    Core (r, c) computes out[4096r:4096(r+1), 256c:256(c+1)].
  * Host prep is pure layout (transpose/reshape/slice): x is passed
    transposed and tiled so every device DMA is a large contiguous
    block with 8-16KB contiguous per SBUF partition row (the measured
    DMA-efficiency knee on this part).
  * On-device per core: the spline g-axis reduce runs on the Vector
    engine, weights are cast to bf16, x is cast (bf16) + silu'd (Scalar
    engine), and the two matmuls are fused into one K=2048 bf16
    accumulation per PSUM tile on the Tensor engine (f32 accumulate).
  * Output is written bf16 (intermediate rounding only; the f32
    result of the accumulation is rounded once) and upcast to f32 on
    gather. End-to-end relative error vs the f32 reference is ~3e-3.
"""
import sys

for _p in ("/opt/trn_rl_repo",):
    if _p not in sys.path:
        sys.path.insert(0, _p)

import numpy as np

import concourse.bass as bass  # noqa: F401  (bass must import before mybir use)
import concourse.mybir as mybir
import concourse.tile as tile
from concourse import bacc
from concourse.bass_utils import run_bass_kernel_spmd

P = 128
IN_F = 1024
G = 8
N_CORES = 8
R_SPLIT = 2
C_SPLIT = 4
B_LOC = 8192 // R_SPLIT      # 4096 batch rows per core
O_LOC = 1024 // C_SPLIT      # 256 out features per core
KT = IN_F // P               # 8 k-tiles over in_features
M_CHUNK = 512
N_CHUNKS = B_LOC // M_CHUNK  # 8
J_SUB = M_CHUNK // P         # 4

F32 = mybir.dt.float32
BF16 = mybir.dt.bfloat16
AF = mybir.ActivationFunctionType

_compiled = None


def _build_kernel():
    nc = bacc.Bacc(None, target_bir_lowering=False, num_devices=N_CORES)
    xt = nc.dram_tensor("xt", [N_CHUNKS, P, KT, M_CHUNK], F32, kind="ExternalInput")
    bt = nc.dram_tensor("bt", [P, KT, O_LOC], F32, kind="ExternalInput")
    st = nc.dram_tensor("st", [KT, P, G, O_LOC], F32, kind="ExternalInput")
    out = nc.dram_tensor("out", [N_CHUNKS, P, J_SUB, O_LOC], BF16,
                         kind="ExternalOutput")

    with tile.TileContext(nc) as tc:
        with (
            tc.tile_pool(name="wconst", bufs=1) as wconst,
            tc.tile_pool(name="wstage", bufs=2) as wstage,
            tc.tile_pool(name="xstage", bufs=4) as xstage,
            tc.tile_pool(name="xcat", bufs=4) as xcat,
            tc.tile_pool(name="psum", bufs=8, space="PSUM") as psum,
            tc.tile_pool(name="opool", bufs=6) as opool,
        ):
            # ---- base weights -> bf16 k-tiles ----
            bstage = wconst.tile([P, KT, O_LOC], F32, name="bstage")
            nc.sync.dma_start(bstage[:], bt[:])
            wb_bf = []
            for t in range(KT):
                wbb = wconst.tile([P, O_LOC], BF16, name=f"wbb{t}")
                nc.vector.tensor_copy(wbb[:], bstage[:, t])
                wb_bf.append(wbb)

            # ---- spline weight: g-sum on DVE, then bf16 ----
            ws_bf = []
            for t in range(KT):
                stg = wstage.tile([P, G, O_LOC], F32, name="stg", tag="stg")
                nc.sync.dma_start(stg[:], st[t])
                acc = wstage.tile([P, O_LOC], F32, name="wsac", tag="wsac")
                h1 = wstage.tile([P, O_LOC], F32, name="wsh1", tag="wsh1")
                nc.vector.tensor_add(acc[:], stg[:, 0], stg[:, 1])
                nc.vector.tensor_add(h1[:], stg[:, 2], stg[:, 3])
                nc.vector.tensor_add(acc[:], acc[:], h1[:])
                nc.vector.tensor_add(h1[:], stg[:, 4], stg[:, 5])
                nc.vector.tensor_add(acc[:], acc[:], h1[:])
                nc.vector.tensor_add(h1[:], stg[:, 6], stg[:, 7])
                nc.vector.tensor_add(acc[:], acc[:], h1[:])
                wsb = wconst.tile([P, O_LOC], BF16, name=f"wsb{t}")
                nc.vector.tensor_copy(wsb[:], acc[:])
                ws_bf.append(wsb)

            # ---- stream batch chunks: cast + silu + fused K=2048 matmul ----
            for ch in range(N_CHUNKS):
                xf = xstage.tile([P, KT, M_CHUNK], F32, name="xf", tag="xf")
                nc.sync.dma_start(xf[:], xt[ch])
                xb = xcat.tile([P, KT, M_CHUNK], BF16, name="xb", tag="xb")
                nc.vector.tensor_copy(xb[:], xf[:])
                sb = xcat.tile([P, KT, M_CHUNK], BF16, name="sb", tag="sb")
                nc.scalar.activation(sb[:], xf[:], AF.Silu)

                ot = opool.tile([P, J_SUB, O_LOC], BF16, name="ot")
                for j in range(J_SUB):
                    pt = psum.tile([P, O_LOC], F32, name="pt")
                    js = slice(P * j, P * (j + 1))
                    for k in range(KT):
                        nc.tensor.matmul(
                            pt[:], xb[:, k, js], wb_bf[k][:],
                            start=(k == 0), stop=False,
                        )
                    for k in range(KT):
                        nc.tensor.matmul(
                            pt[:], sb[:, k, js], ws_bf[k][:],
                            start=False, stop=(k == KT - 1),
                        )
                    nc.any.tensor_copy(ot[:, j], pt[:])
                nc.sync.dma_start(out[ch], ot[:])
    nc.compile()
    return nc


def _get_compiled():
    global _compiled
    if _compiled is None:
        _compiled = _build_kernel()
    return _compiled


def _shard_inputs(x, base_weight, spline_weight):
    """Full inputs -> 8 per-core in_maps (pure layout transforms)."""
    x = np.ascontiguousarray(np.asarray(x, dtype=np.float32))
    base_weight = np.ascontiguousarray(np.asarray(base_weight, dtype=np.float32))
    spline_weight = np.ascontiguousarray(np.asarray(spline_weight, dtype=np.float32))

    xt_full = np.ascontiguousarray(x.T)                     # [1024, 8192]
    btf = np.ascontiguousarray(base_weight.T)               # [1024, 1024]
    in_maps = []
    for core in range(N_CORES):
        r, c = divmod(core, C_SPLIT)
        osl = slice(O_LOC * c, O_LOC * (c + 1))
        xs = xt_full[:, B_LOC * r:B_LOC * (r + 1)]          # [1024, 4096]
        # [ch, p, it, b]: one contiguous 2MB block per chunk, 16KB rows
        xs6 = (xs.reshape(KT, P, N_CHUNKS, M_CHUNK)
                 .transpose(2, 1, 0, 3))
        btc = btf[:, osl].reshape(KT, P, O_LOC).transpose(1, 0, 2)
        stc = (spline_weight[osl]                      # [256 o, 1024 i, 8 g]
               .transpose(1, 2, 0)                     # [1024 i, 8 g, 256 o]
               .reshape(KT, P, G, O_LOC))
        in_maps.append({
            "xt": np.ascontiguousarray(xs6),
            "bt": np.ascontiguousarray(btc),
            "st": np.ascontiguousarray(stc),
        })
    return in_maps


def _gather_output(results):
    out = np.empty((8192, 1024), dtype=np.float32)
    for core in range(N_CORES):
        r, c = divmod(core, C_SPLIT)
        oc = results[core]["out"].astype(np.float32)   # [8 ch, 128 p, 4 j, 256 o]
        oc = oc.transpose(0, 2, 1, 3).reshape(B_LOC, O_LOC)
        out[B_LOC * r:B_LOC * (r + 1), O_LOC * c:O_LOC * (c + 1)] = oc
    return out


def run(trace=False, **inputs):
    """Run on the 8 NeuronCores; returns (out, BassKernelResults)."""
    nc = _get_compiled()
    in_maps = _shard_inputs(**inputs)
    res = run_bass_kernel_spmd(
        nc, in_maps, core_ids=list(range(N_CORES)), trace=trace)
    return _gather_output(res.results), res


def kernel(**inputs) -> np.ndarray:
    out, _ = run(trace=False, **inputs)
    return out
